# revision 2
# baseline (speedup 1.0000x reference)
"""Trainium2 Bass kernel for causal self-attention (B=4, T=2048, C=2048, H=16).

Sharding: 8 cores = DP4 (batch) x TP2 (8 heads each). Each core:
  P1  qk' = (x @ Wqk)^T computed directly in [j, t] layout (j = head-dim rows)
  P2  v   = x @ Wv in natural [t, j] layout (stationary = xT tiles)
  P3  per head: s_T = k'^T q' -> exp -> causal mask -> av + replicated-ones den
      -> y'_h = o' * recip(den)    (y' kept in [c', t] layout for proj)
  P4  out_partial[t, n] = y'^T @ Wp (stationary = y' tiles) + bias (even core)
  P5  ReduceScatter(add) over core pairs -> each core owns half the t rows.

All matmuls bf16 with fp32 PSUM accumulation; softmax in fp32 on ACT/DVE.
Host side: shard/cast/transpose inputs, assemble output.
"""
import os
import math
import numpy as np
import ml_dtypes

import concourse.bass as bass
import concourse.bacc as bacc
import concourse.mybir as mybir
import concourse.tile as tile

F32 = mybir.dt.float32
BF16 = mybir.dt.bfloat16
AF = mybir.ActivationFunctionType

D = 128          # head dim (fixed: partition size)
N_CORES = 8
PAIRS = [[0, 1], [2, 3], [4, 5], [6, 7]]


class Cfg:
    def __init__(self, T=2048, H_TOT=16, HPC=8, B=4):
        self.T = T                    # sequence length
        self.H_TOT = H_TOT            # total heads
        self.HPC = HPC                # heads per core
        self.B = B
        self.C = H_TOT * D            # model dim
        self.CP = HPC * D             # per-core head cols
        self.TCH = 512                # ti chunk width
        assert T % self.TCH == 0 and T % D == 0


def build_kernel(cfg: Cfg):
    T, C, CP, HPC, TCH = cfg.T, cfg.C, cfg.CP, cfg.HPC, cfg.TCH
    NC_CH = C // D                # c-chunks (contraction)
    NJB = 2 * HPC                 # qk' j-blocks (q heads then k heads)
    NTB = T // D                  # t-blocks
    NIC = T // TCH                # ti chunks
    NTR = T // 512                # t-ranges for P1 moving dim
    NPR = C // 512                # proj n-ranges
    scale = 1.0 / math.sqrt(D)

    nc = bacc.Bacc()
    xT = nc.declare_dram_parameter("xT", [C, T], BF16, isOutput=False)
    wqk = nc.declare_dram_parameter("wqk", [C, 2 * CP], BF16, isOutput=False)
    wv = nc.declare_dram_parameter("wv", [C, CP], BF16, isOutput=False)
    wp = nc.declare_dram_parameter("wp", [CP, C], BF16, isOutput=False)
    bqk = nc.declare_dram_parameter("bqk", [D, NJB], F32, isOutput=False)
    bv = nc.declare_dram_parameter("bv", [1, CP], BF16, isOutput=False)
    bp = nc.declare_dram_parameter("bp", [1, C], BF16, isOutput=False)
    masks = nc.declare_dram_parameter("masks", [D, 4 * TCH], BF16, isOutput=False)
    out_ext = nc.declare_dram_parameter("out", [T // 2, C], F32, isOutput=True)

    qk_dram = nc.dram_tensor("qk_dram", [HPC, 2, D, T], BF16)
    partial_dram = nc.dram_tensor("partial_dram", [T, C], F32)
    rs_out = nc.dram_tensor("rs_out", [T // 2, C], F32)

    with tile.TileContext(nc) as tc:
        with (
            tc.tile_pool(name="const", bufs=1) as constp,
            tc.tile_pool(name="vres", bufs=1) as vres,
            tc.tile_pool(name="yres", bufs=1) as yres,
        ):
            bqk_t = constp.tile([D, NJB], F32, name="bqk_t")
            nc.sync.dma_start(bqk_t[:], bqk[:, :])
            bv_t = constp.tile([1, CP], BF16, name="bv_t")
            nc.sync.dma_start(bv_t[:], bv[:, :])
            bp_t = constp.tile([1, C], BF16, name="bp_t")
            nc.sync.dma_start(bp_t[:], bp[:, :])
            # masks: partition dim must be D -> load as [D, 4*TCH]
            mask_sb = constp.tile([D, 4 * TCH], BF16, name="mask_sb")
            nc.sync.dma_start(mask_sb[:], masks[:, :])
            ones_sq = constp.tile([D, D], BF16, name="ones_sq")
            nc.vector.memset(ones_sq[:], 1.0)
            ones_row = constp.tile([1, D], BF16, name="ones_row")
            nc.vector.memset(ones_row[:], 1.0)

            # resident xT tiles (bf16), one per c-chunk; freed after P2
            xtp_ctx = tc.tile_pool(name="xtp", bufs=1)
            xtp = xtp_ctx.__enter__()
            xt = []
            for c in range(NC_CH):
                t = xtp.tile([D, T], BF16, name=f"xt{c}")
                nc.sync.dma_start(t[:], xT[c * D:(c + 1) * D, :])
                xt.append(t)

            # ---------------- P1: qk' ----------------
            with (
                tc.tile_pool(name="wqkp", bufs=2 * NC_CH) as wqkp,
                tc.tile_pool(name="pq", bufs=8, space="PSUM") as pqp,
                tc.tile_pool(name="qkst", bufs=4) as qkst,
            ):
                for jb in range(NJB):
                    wts = []
                    for c in range(NC_CH):
                        wt = wqkp.tile([D, D], BF16, name="wt", tag="wt")
                        nc.sync.dma_start(
                            wt[:], wqk[c * D:(c + 1) * D, jb * D:(jb + 1) * D])
                        wts.append(wt)
                    ps = [pqp.tile([D, 512], F32, name="pq", tag="pq")
                          for _ in range(NTR)]
                    for c in range(NC_CH):
                        for tr in range(NTR):
                            nc.tensor.matmul(
                                ps[tr][:], wts[c][:],
                                xt[c][:, tr * 512:(tr + 1) * 512],
                                start=(c == 0), stop=(c == NC_CH - 1))
                    st = qkst.tile([D, T], BF16, name="qks", tag="qks")
                    for tr in range(NTR):
                        nc.vector.tensor_scalar_add(
                            st[:, tr * 512:(tr + 1) * 512], ps[tr][:],
                            bqk_t[:, jb:jb + 1])
                    nc.sync.dma_start(qk_dram[jb // 2, jb % 2, :, :], st[:])

            # ---------------- P2: v ----------------
            v_sb = []
            with (
                tc.tile_pool(name="wvp", bufs=1) as wvp,
                tc.tile_pool(name="pv", bufs=4, space="PSUM") as pvp,
            ):
                wv_t = []
                for c in range(NC_CH):
                    t = wvp.tile([D, CP], BF16, name=f"wv{c}")
                    nc.sync.dma_start(t[:], wv[c * D:(c + 1) * D, :])
                    wv_t.append(t)
                NVR = CP // 512 if CP >= 512 else 1
                VRW = min(512, CP)
                for tb in range(NTB):
                    ps = [pvp.tile([D, VRW], F32, name="pv", tag="pv")
                          for _ in range(NVR)]
                    for c in range(NC_CH):
                        for vr in range(NVR):
                            nc.tensor.matmul(
                                ps[vr][:], xt[c][:, tb * D:(tb + 1) * D],
                                wv_t[c][:, vr * VRW:(vr + 1) * VRW],
                                start=(c == 0), stop=False)
                    for vr in range(NVR):
                        nc.tensor.matmul(
                            ps[vr][:], ones_row[:],
                            bv_t[:, vr * VRW:(vr + 1) * VRW],
                            start=False, stop=True)
                    vt = vres.tile([D, CP], BF16, name=f"v{tb}")
                    for vr in range(NVR):
                        nc.vector.tensor_copy(
                            vt[:, vr * VRW:(vr + 1) * VRW], ps[vr][:])
                    v_sb.append(vt)
            xtp_ctx.__exit__(None, None, None)

            # ---------------- P3: attention ----------------
            y_sb = []
            with (
                tc.tile_pool(name="qkio", bufs=2) as qkio,
                tc.tile_pool(name="attp", bufs=6) as attp,
                tc.tile_pool(name="ps_s", bufs=3, space="PSUM") as ps_s,
                tc.tile_pool(name="ps_o", bufs=2, space="PSUM") as ps_o,
                tc.tile_pool(name="ps_d", bufs=2, space="PSUM") as ps_d,
                tc.tile_pool(name="normp", bufs=4) as normp,
            ):
                for h in range(HPC):
                    qk_sb = qkio.tile([D, 2 * T], BF16, name="qk_sb", tag="qkio")
                    qp = qk_sb[:, 0:T]
                    kp = qk_sb[:, T:2 * T]
                    nc.sync.dma_start(qp, qk_dram[h, 0, :, :])
                    nc.sync.dma_start(kp, qk_dram[h, 1, :, :])
                    yt = yres.tile([D, T], BF16, name=f"y{h}")
                    for ic in range(NIC):
                        ti0 = ic * TCH
                        ntk = (ti0 + TCH) // D
                        po = ps_o.tile([D, TCH], F32, name="po", tag="po")
                        pd = ps_d.tile([D, TCH], F32, name="pd", tag="pd")
                        for tk in range(ntk):
                            s_p = ps_s.tile([D, TCH], F32, name="sp", tag="sp")
                            nc.tensor.matmul(
                                s_p[:], kp[:, tk * D:(tk + 1) * D],
                                qp[:, ti0:ti0 + TCH], start=True, stop=True)
                            kdiag = tk - (ntk - TCH // D)
                            if kdiag >= 0:
                                nc.vector.tensor_add(
                                    s_p[:], s_p[:],
                                    mask_sb[:, kdiag * TCH:(kdiag + 1) * TCH])
                            att = attp.tile([D, TCH], BF16, name="att", tag="att")
                            nc.scalar.activation(att[:], s_p[:], AF.Exp,
                                                 bias=0.0, scale=scale)
                            nc.tensor.matmul(
                                po[:], v_sb[tk][:, h * D:(h + 1) * D], att[:],
                                start=(tk == 0), stop=(tk == ntk - 1))
                            nc.tensor.matmul(
                                pd[:], ones_sq[:], att[:],
                                start=(tk == 0), stop=(tk == ntk - 1))
                        rec = normp.tile([D, TCH], F32, name="rec", tag="rec")
                        nc.vector.reciprocal(rec[:], pd[:])
                        nc.vector.tensor_mul(yt[:, ti0:ti0 + TCH], po[:], rec[:])
                    y_sb.append(yt)

            # ---------------- P4: proj partials ----------------
            with (
                tc.tile_pool(name="wpp", bufs=1) as wpp,
                tc.tile_pool(name="pp", bufs=8, space="PSUM") as ppp,
                tc.tile_pool(name="post", bufs=3) as post,
            ):
                wp_t = []
                for c in range(HPC):
                    t = wpp.tile([D, C], BF16, name=f"wp{c}")
                    nc.sync.dma_start(t[:], wp[c * D:(c + 1) * D, :])
                    wp_t.append(t)
                for tb in range(NTB):
                    ps = [ppp.tile([D, 512], F32, name="pp", tag="pp")
                          for _ in range(NPR)]
                    for c in range(HPC):
                        for nr in range(NPR):
                            nc.tensor.matmul(
                                ps[nr][:], y_sb[c][:, tb * D:(tb + 1) * D],
                                wp_t[c][:, nr * 512:(nr + 1) * 512],
                                start=(c == 0), stop=False)
                    for nr in range(NPR):
                        nc.tensor.matmul(
                            ps[nr][:], ones_row[:],
                            bp_t[:, nr * 512:(nr + 1) * 512],
                            start=False, stop=True)
                    st = post.tile([D, C], F32, name="pst", tag="pst")
                    for nr in range(NPR):
                        nc.vector.tensor_copy(
                            st[:, nr * 512:(nr + 1) * 512], ps[nr][:])
                    nc.sync.dma_start(partial_dram[tb * D:(tb + 1) * D, :], st[:])

            # ---------------- P5: ReduceScatter pairs ----------------
            nc.gpsimd.collective_compute(
                "ReduceScatter",
                mybir.AluOpType.add,
                ins=[partial_dram[:, :]],
                outs=[rs_out[:, :]],
                replica_groups=PAIRS,
            )
            nc.sync.dma_start(out_ext[:, :], rs_out[:, :])
    nc.finalize()
    return nc


def _prep_inputs(cfg: Cfg, x, w_attn, b_attn, w_proj, b_proj):
    """Host-side shard/cast. Returns in_maps (list of dicts per core)."""
    T, C, CP, HPC = cfg.T, cfg.C, cfg.CP, cfg.HPC
    bf = ml_dtypes.bfloat16
    wq = w_attn[:, 0:C]
    wk = w_attn[:, C:2 * C]
    wvf = w_attn[:, 2 * C:3 * C]
    bq, bk, bvf = b_attn[0:C], b_attn[C:2 * C], b_attn[2 * C:3 * C]

    masks = np.zeros((D, 4 * cfg.TCH), dtype=bf)
    f = np.arange(cfg.TCH)[None, :]
    p = np.arange(D)[:, None]
    for k in range(4):
        keep = (f - p >= 128 * k)
        masks[:, k * cfg.TCH:(k + 1) * cfg.TCH] = np.where(
            keep, 0.0, -30000.0).astype(bf)

    in_maps = []
    for core in range(N_CORES):
        b = core // 2
        g = core % 2
        h0 = g * HPC * D            # first col of this head group
        sl = slice(h0, h0 + CP)
        xTc = np.ascontiguousarray(x[b].T).astype(bf)
        wqk_cols = []
        for h in range(HPC):
            hs = slice(h0 + h * D, h0 + (h + 1) * D)
            wqk_cols.append(wq[:, hs])
            wqk_cols.append(wk[:, hs])
        wqk_c = np.concatenate(wqk_cols, axis=1).astype(bf)
        wv_c = wvf[:, sl].astype(bf)
        wp_c = w_proj[sl, :].astype(bf)
        bqk_cols = []
        for h in range(HPC):
            hs = slice(h0 + h * D, h0 + (h + 1) * D)
            bqk_cols.append(bq[hs])
            bqk_cols.append(bk[hs])
        bqk_c = np.ascontiguousarray(np.stack(bqk_cols, axis=1)).astype(np.float32)
        in_maps.append({
            "xT": xTc,
            "wqk": wqk_c,
            "wv": wv_c,
            "wp": wp_c,
            "bqk": bqk_c,
            "bv": bvf[sl].reshape(1, CP).astype(bf),
            "bp": (b_proj * (1.0 - g)).reshape(1, C).astype(bf),
            "masks": masks,
        })
    return in_maps


_CFG = Cfg()


def kernel(x, w_attn, b_attn, w_proj, b_proj, _trace=False, _cfg=None):
    from concourse.bass_utils import run_bass_kernel_spmd
    cfg = _cfg or _CFG
    x = np.asarray(x, dtype=np.float32)
    w_attn = np.asarray(w_attn, dtype=np.float32)
    b_attn = np.asarray(b_attn, dtype=np.float32)
    w_proj = np.asarray(w_proj, dtype=np.float32)
    b_proj = np.asarray(b_proj, dtype=np.float32)

    in_maps = _prep_inputs(cfg, x, w_attn, b_attn, w_proj, b_proj)
    nc = build_kernel(cfg)
    res = run_bass_kernel_spmd(nc, in_maps, list(range(N_CORES)), trace=_trace)
    outs = []
    for b in range(cfg.B):
        top = res.results[2 * b]["out"]
        bot = res.results[2 * b + 1]["out"]
        outs.append(np.concatenate([top, bot], axis=0))
    full = np.stack(outs, axis=0).astype(np.float32)
    if _trace:
        kernel.last_exec_time_ns = res.exec_time_ns
        kernel.last_mean_exec_time_ns = res.mean_exec_time_ns
        kernel.last_scope_times = res.per_core_scope_times
        kernel.last_trace_path = (res.instructions_and_trace[1]
                                  if res.instructions_and_trace else None)
        kernel.last_insts = (res.instructions_and_trace[0]
                             if res.instructions_and_trace else None)
    return full



# revision 11
# speedup vs baseline: 1.0696x; 1.0696x over previous
"""Trainium2 Bass kernel for causal self-attention (B=4, T=2048, C=2048, H=16).

Sharding: 8 cores = DP4 (batch) x TP2 (8 heads each). Each core:
  P1  qk' = (x @ Wqk)^T computed directly in [j, t] layout (j = head-dim rows),
      kept resident in SBUF (no DRAM round trip).
  P2  v   = x @ Wv in natural [t, j] layout (stationary = xT tiles), with the
      Wv weights streamed per 512-col range so xT + qk' + v all fit in SBUF.
  P3/P4 interleaved per query-chunk ic (512 queries):
      P3  per head: s = k'^T q' -> mask -> exp -> att; po += v att;
          pd += ones att; y = po * recip(pd)   (y kept [c', t] for proj)
      P4  partial[tb, :] = y'^T @ Wp + bias (even core), stored bf16
      RS  chunked ReduceScatter(add) of this quarter's rows over the core
          pair, output in Shared DRAM, copied to the external output. The
          collectives pipeline behind the next quarter's compute.

All matmuls bf16 with fp32 PSUM accumulation; softmax in fp32 on ACT/DVE.
Host side: shard/cast/transpose inputs, assemble output (bf16 -> f32).
"""
import os
import math
import numpy as np
import ml_dtypes

import concourse.bass as bass
import concourse.bacc as bacc
import concourse.mybir as mybir
import concourse.tile as tile

F32 = mybir.dt.float32
BF16 = mybir.dt.bfloat16
AF = mybir.ActivationFunctionType

D = 128          # head dim (fixed: partition size)
N_CORES = 8
PAIRS = [[0, 1], [2, 3], [4, 5], [6, 7]]


class Cfg:
    def __init__(self, T=2048, H_TOT=16, HPC=8, B=4):
        self.T = T                    # sequence length
        self.H_TOT = H_TOT            # total heads
        self.HPC = HPC                # heads per core
        self.B = B
        self.C = H_TOT * D            # model dim
        self.CP = HPC * D             # per-core head cols
        self.TCH = 512                # ti chunk width
        assert T % self.TCH == 0 and T % D == 0


def build_kernel(cfg: Cfg):
    T, C, CP, HPC, TCH = cfg.T, cfg.C, cfg.CP, cfg.HPC, cfg.TCH
    NC_CH = C // D                # c-chunks (contraction)
    NJB = 2 * HPC                 # qk' j-blocks (q/k interleaved per head)
    NTB = T // D                  # t-blocks
    NIC = T // TCH                # ti chunks (query quarters)
    NTR = T // 512                # t-ranges for P1 moving dim
    NPR = C // 512                # proj n-ranges
    VRW = 256                     # v col range width
    NVR = CP // VRW               # v col ranges
    TBQ = TCH // D                # t-blocks per quarter
    RQ = TCH // 2                 # rows each core owns per quarter after RS
    scale = 1.0 / math.sqrt(D)

    nc = bacc.Bacc()
    xT = nc.declare_dram_parameter("xT", [C, T], BF16, isOutput=False)
    wqk = nc.declare_dram_parameter("wqk", [C, 2 * CP], BF16, isOutput=False)
    wv = nc.declare_dram_parameter("wv", [C, CP], BF16, isOutput=False)
    wp = nc.declare_dram_parameter("wp", [CP, C], BF16, isOutput=False)
    bqk = nc.declare_dram_parameter("bqk", [D, NJB], F32, isOutput=False)
    bv = nc.declare_dram_parameter("bv", [1, CP], BF16, isOutput=False)
    bp = nc.declare_dram_parameter("bp", [1, C], BF16, isOutput=False)
    masks = nc.declare_dram_parameter("masks", [D, 4 * TCH], BF16, isOutput=False)
    out_ext = nc.declare_dram_parameter("out", [NIC * RQ, C], BF16, isOutput=True)

    partial_q = [nc.dram_tensor(f"partial_{q}", [TCH, C], BF16)
                 for q in range(NIC)]
    rs_q = [nc.dram_tensor(f"rs_{q}", [RQ, C], BF16)
            for q in range(NIC)]

    with tile.TileContext(nc) as tc:
        with (
            tc.tile_pool(name="const", bufs=1) as constp,
            tc.tile_pool(name="vres", bufs=1) as vres,
            tc.tile_pool(name="yres", bufs=16) as yres,
            tc.tile_pool(name="qkres", bufs=1) as qkres,
        ):
            # ---------------- P1: qk' ----------------
            # first j-block's weights before the big x load so the PE can
            # start as soon as xT chunk 0 lands (pool open order is LIFO:
            # xtp outlives wqkp)
            xtp_ctx = tc.tile_pool(name="xtp", bufs=1)
            xtp = xtp_ctx.__enter__()
            wqkp_ctx = tc.tile_pool(name="wqkp", bufs=2 * NC_CH)
            wqkp = wqkp_ctx.__enter__()

            def load_wt(jb):
                wts = []
                for c in range(NC_CH):
                    wt = wqkp.tile([D, D], BF16, name="wt", tag="wt")
                    nc.sync.dma_start(
                        wt[:], wqk[c * D:(c + 1) * D, jb * D:(jb + 1) * D])
                    wts.append(wt)
                return wts

            wts0 = load_wt(0)

            bqk_t = constp.tile([D, NJB], F32, name="bqk_t")
            nc.sync.dma_start(bqk_t[:], bqk[:, :])

            # resident xT tiles (bf16), one per c-chunk; freed after P2
            xt = []
            for c in range(NC_CH):
                t = xtp.tile([D, T], BF16, name=f"xt{c}")
                nc.sync.dma_start(t[:], xT[c * D:(c + 1) * D, :])
                xt.append(t)

            bv_t = constp.tile([1, CP], BF16, name="bv_t")
            nc.sync.dma_start(bv_t[:], bv[:, :])
            bp_t = constp.tile([1, C], BF16, name="bp_t")
            nc.sync.dma_start(bp_t[:], bp[:, :])
            # masks: partition dim must be D -> load as [D, 4*TCH]
            mask_sb = constp.tile([D, 4 * TCH], BF16, name="mask_sb")
            nc.sync.dma_start(mask_sb[:], masks[:, :])
            ones_sq = constp.tile([D, D], BF16, name="ones_sq")
            nc.vector.memset(ones_sq[:], 1.0)
            ones_row = constp.tile([1, D], BF16, name="ones_row")
            nc.vector.memset(ones_row[:], 1.0)

            # persistent qk' tiles [D, T] per j-block
            qk_sb = [qkres.tile([D, T], BF16, name=f"qk{jb}")
                     for jb in range(NJB)]
            with tc.tile_pool(name="pq", bufs=8, space="PSUM") as pqp:
                for jb in range(NJB):
                    wts = wts0 if jb == 0 else load_wt(jb)
                    ps = [pqp.tile([D, 512], F32, name="pq", tag="pq")
                          for _ in range(NTR)]
                    for c in range(NC_CH):
                        for tr in range(NTR):
                            nc.tensor.matmul(
                                ps[tr][:], wts[c][:],
                                xt[c][:, tr * 512:(tr + 1) * 512],
                                start=(c == 0), stop=(c == NC_CH - 1))
                    for tr in range(NTR):
                        nc.vector.tensor_scalar_add(
                            qk_sb[jb][:, tr * 512:(tr + 1) * 512], ps[tr][:],
                            bqk_t[:, jb:jb + 1])
            wqkp_ctx.__exit__(None, None, None)

            # ---------------- P2: v ----------------
            # v_sb[tb] is [t, j]; Wv streamed one 512-col range at a time
            v_sb = [vres.tile([D, CP], BF16, name=f"v{tb}")
                    for tb in range(NTB)]
            for vr in range(NVR):
                with (
                    tc.tile_pool(name="wvp", bufs=1) as wvp,
                    tc.tile_pool(name="pv", bufs=4, space="PSUM") as pvp,
                ):
                    wv_t = wvp.tile([D, NC_CH * VRW], BF16, name=f"wv{vr}")
                    for c in range(NC_CH):
                        nc.sync.dma_start(
                            wv_t[:, c * VRW:(c + 1) * VRW],
                            wv[c * D:(c + 1) * D, vr * VRW:(vr + 1) * VRW])
                    for tb in range(NTB):
                        pv = pvp.tile([D, VRW], F32, name="pv", tag="pv")
                        for c in range(NC_CH):
                            nc.tensor.matmul(
                                pv[:], xt[c][:, tb * D:(tb + 1) * D],
                                wv_t[:, c * VRW:(c + 1) * VRW],
                                start=(c == 0), stop=False)
                        nc.tensor.matmul(
                            pv[:], ones_row[:],
                            bv_t[:, vr * VRW:(vr + 1) * VRW],
                            start=False, stop=True)
                        nc.vector.tensor_copy(
                            v_sb[tb][:, vr * VRW:(vr + 1) * VRW], pv[:])
            xtp_ctx.__exit__(None, None, None)

            # ---------------- P3 + P4 + RS interleaved per quarter --------
            with (
                tc.tile_pool(name="wpp", bufs=1) as wpp,
                tc.tile_pool(name="attp", bufs=6) as attp,
                tc.tile_pool(name="ps_s", bufs=2, space="PSUM") as ps_s,
                tc.tile_pool(name="ps_o", bufs=2, space="PSUM") as ps_o,
                tc.tile_pool(name="ps_d", bufs=2, space="PSUM") as ps_d,
                tc.tile_pool(name="pp", bufs=2, space="PSUM") as ppp,
                tc.tile_pool(name="normp", bufs=4) as normp,
                tc.tile_pool(name="post", bufs=3) as post,
            ):
                # prefetch proj weights now that xT is freed
                wp_t = []
                for c in range(HPC):
                    t = wpp.tile([D, C], BF16, name=f"wp{c}")
                    nc.sync.dma_start(t[:], wp[c * D:(c + 1) * D, :])
                    wp_t.append(t)

                for ic in range(NIC):
                    ti0 = ic * TCH
                    ntk = (ti0 + TCH) // D
                    # per-quarter y tiles (consumed by P4 right below)
                    y_sb = [yres.tile([D, TCH], BF16, name="y", tag="y")
                            for _ in range(HPC)]
                    for h in range(HPC):
                        qp = qk_sb[2 * h]
                        kp = qk_sb[2 * h + 1]
                        po = ps_o.tile([D, TCH], F32, name="po", tag="po")
                        pd = ps_d.tile([D, TCH], F32, name="pd", tag="pd")
                        for tk in range(ntk):
                            s_p = ps_s.tile([D, TCH], F32, name="sp", tag="sp")
                            nc.tensor.matmul(
                                s_p[:], kp[:, tk * D:(tk + 1) * D],
                                qp[:, ti0:ti0 + TCH], start=True, stop=True)
                            kdiag = tk - (ntk - TCH // D)
                            if kdiag >= 0:
                                nc.vector.tensor_add(
                                    s_p[:], s_p[:],
                                    mask_sb[:, kdiag * TCH:(kdiag + 1) * TCH])
                            att = attp.tile([D, TCH], BF16, name="att",
                                            tag="att")
                            nc.scalar.activation(att[:], s_p[:], AF.Exp,
                                                 bias=0.0, scale=scale)
                            nc.tensor.matmul(
                                po[:], v_sb[tk][:, h * D:(h + 1) * D], att[:],
                                start=(tk == 0), stop=(tk == ntk - 1))
                            nc.tensor.matmul(
                                pd[:], ones_sq[:], att[:],
                                start=(tk == 0), stop=(tk == ntk - 1))
                        rec = normp.tile([D, TCH], F32, name="rec", tag="rec")
                        nc.vector.reciprocal(rec[:], pd[:])
                        nc.vector.tensor_mul(y_sb[h][:], po[:], rec[:])

                    # ---- P4 for this quarter ----
                    for tq in range(TBQ):
                        pst = post.tile([D, C], BF16, name="pst", tag="pst")
                        for nr in range(NPR):
                            pp = ppp.tile([D, 512], F32, name="pp", tag="pp")
                            for c in range(HPC):
                                nc.tensor.matmul(
                                    pp[:], y_sb[c][:, tq * D:(tq + 1) * D],
                                    wp_t[c][:, nr * 512:(nr + 1) * 512],
                                    start=(c == 0), stop=False)
                            nc.tensor.matmul(
                                pp[:], ones_row[:],
                                bp_t[:, nr * 512:(nr + 1) * 512],
                                start=False, stop=True)
                            nc.vector.tensor_copy(
                                pst[:, nr * 512:(nr + 1) * 512], pp[:])
                        nc.sync.dma_start(
                            partial_q[ic][tq * D:(tq + 1) * D, :], pst[:])

                    # ---- chunked ReduceScatter for this quarter ----
                    nc.gpsimd.collective_compute(
                        "ReduceScatter",
                        mybir.AluOpType.add,
                        ins=[partial_q[ic][:, :]],
                        outs=[rs_q[ic][:, :]],
                        replica_groups=PAIRS,
                    )
                    nc.sync.dma_start(
                        out_ext[ic * RQ:(ic + 1) * RQ, :], rs_q[ic][:, :])
    nc.finalize()
    return nc


def _prep_inputs(cfg: Cfg, x, w_attn, b_attn, w_proj, b_proj):
    """Host-side shard/cast. Returns in_maps (list of dicts per core)."""
    T, C, CP, HPC = cfg.T, cfg.C, cfg.CP, cfg.HPC
    bf = ml_dtypes.bfloat16
    wq = w_attn[:, 0:C]
    wk = w_attn[:, C:2 * C]
    wvf = w_attn[:, 2 * C:3 * C]
    bq, bk, bvf = b_attn[0:C], b_attn[C:2 * C], b_attn[2 * C:3 * C]

    masks = np.zeros((D, 4 * cfg.TCH), dtype=bf)
    f = np.arange(cfg.TCH)[None, :]
    p = np.arange(D)[:, None]
    for k in range(4):
        keep = (f - p >= 128 * k)
        masks[:, k * cfg.TCH:(k + 1) * cfg.TCH] = np.where(
            keep, 0.0, -30000.0).astype(bf)

    in_maps = []
    for core in range(N_CORES):
        b = core // 2
        g = core % 2
        h0 = g * HPC * D            # first col of this head group
        sl = slice(h0, h0 + CP)
        xTc = np.ascontiguousarray(x[b].T).astype(bf)
        wqk_cols = []
        for h in range(HPC):
            hs = slice(h0 + h * D, h0 + (h + 1) * D)
            wqk_cols.append(wq[:, hs])
            wqk_cols.append(wk[:, hs])
        wqk_c = np.concatenate(wqk_cols, axis=1).astype(bf)
        wv_c = wvf[:, sl].astype(bf)
        wp_c = w_proj[sl, :].astype(bf)
        bqk_cols = []
        for h in range(HPC):
            hs = slice(h0 + h * D, h0 + (h + 1) * D)
            bqk_cols.append(bq[hs])
            bqk_cols.append(bk[hs])
        bqk_c = np.ascontiguousarray(np.stack(bqk_cols, axis=1)).astype(np.float32)
        in_maps.append({
            "xT": xTc,
            "wqk": wqk_c,
            "wv": wv_c,
            "wp": wp_c,
            "bqk": bqk_c,
            "bv": bvf[sl].reshape(1, CP).astype(bf),
            "bp": (b_proj * (1.0 - g)).reshape(1, C).astype(bf),
            "masks": masks,
        })
    return in_maps


_CFG = Cfg()


def kernel(x, w_attn, b_attn, w_proj, b_proj, _trace=False, _cfg=None):
    from concourse.bass_utils import run_bass_kernel_spmd
    cfg = _cfg or _CFG
    x = np.asarray(x, dtype=np.float32)
    w_attn = np.asarray(w_attn, dtype=np.float32)
    b_attn = np.asarray(b_attn, dtype=np.float32)
    w_proj = np.asarray(w_proj, dtype=np.float32)
    b_proj = np.asarray(b_proj, dtype=np.float32)

    in_maps = _prep_inputs(cfg, x, w_attn, b_attn, w_proj, b_proj)
    nc = build_kernel(cfg)
    res = run_bass_kernel_spmd(nc, in_maps, list(range(N_CORES)), trace=_trace)
    # out rows per core: quarter q -> rows [512q + 256g, 512q + 256(g+1))
    RQ = cfg.TCH // 2
    outs = []
    for b in range(cfg.B):
        even = res.results[2 * b]["out"].astype(np.float32)
        odd = res.results[2 * b + 1]["out"].astype(np.float32)
        ob = np.empty((cfg.T, cfg.C), dtype=np.float32)
        for q in range(cfg.T // cfg.TCH):
            ob[q * cfg.TCH:q * cfg.TCH + RQ] = even[q * RQ:(q + 1) * RQ]
            ob[q * cfg.TCH + RQ:(q + 1) * cfg.TCH] = odd[q * RQ:(q + 1) * RQ]
        outs.append(ob)
    full = np.stack(outs, axis=0)
    if _trace:
        kernel.last_exec_time_ns = res.exec_time_ns
        kernel.last_mean_exec_time_ns = res.mean_exec_time_ns
        kernel.last_scope_times = res.per_core_scope_times
        kernel.last_trace_path = (res.instructions_and_trace[1]
                                  if res.instructions_and_trace else None)
        kernel.last_insts = (res.instructions_and_trace[0]
                             if res.instructions_and_trace else None)
    return full
